# revision 1
# baseline (speedup 1.0000x reference)
"""MiniBatchDiscrimination kernel, v4: symmetric-pair sharding +
column-tiled paired PE reduction.

Math per core (row block of 64 i's x FD=320 j columns spanning 5 blocks):
  Mt[(o,k), j] = M^T in bf16 (16 partition-tiles), computed on PE.
  For each i:
    relu tiles (DVE, 4x bf16 tensor_scalar):  R_t = max(Mt_t - Mt_t[:,i], 0)
    abs tiles  (ACT offload, 2 tiles):        A_t = |Mt_t - Mt_t[:,i]|
    D[o,j] = sum_A |d| + 2*sum_R relu(d) - (S_j - S_i),  S = sum_k Mt (relu
    tiles only), so D = fold(psumA + psumB) with S_i applied as the Exp bias.
  The 16+1 reduction matmuls are issued as 8 column-tiled PAIRS: tile 2p ->
  PSUM partitions 0:64 (col group 0-1), tile 2p+1 -> partitions 64:128 (col
  group 2-3, tile_position=(0,64)).  The two streams run concurrently on
  disjoint array column groups, nearly halving PE time.  A DVE tensor_add
  folds the halves into a fresh bf16 tile; Exp(scale=-1, bias=-S_i,
  accum_out=rowsum) reads the fold.  The fold is software-pipelined one
  iteration behind the matmuls so its PE wait never stalls the DVE - and it
  doubles as DVE's PE-clock refresh, so slot-reuse waits are pre-observed
  (the walrus here encodes at most ONE sync wait per instruction).
  Column-sum partials (for the partner blocks, by symmetry) are reduced on
  PE from the packed exp tiles once per 8 rows and accumulated in fp32.
"""

import numpy as np
import ml_dtypes
from contextlib import ExitStack

BATCH, IN_FEAT, OUT_FEAT, KERNEL_DIM = 512, 512, 64, 32
N_CORES = 8
ROWB = BATCH // N_CORES          # 64 rows of i per core
OK = OUT_FEAT * KERNEL_DIM       # 2048 flattened (o,k)
NT = OK // 128                   # 16 partition-tiles of (o,k)
NBLK = 5                         # column blocks per core
FD = NBLK * 64                   # 320
POISON = 1.0e4

CHUNK = 16                       # i's per colsum PSUM chunk
SELW = OUT_FEAT
ACT_TILES = (5, 7, 11)           # elementwise tiles computed on ACT as Abs
ADV_BUFS = 56                    # 4 iterations of DVE elementwise tiles

_cache = {}


def _build_nc(split_waits=True):
    import concourse.bass as bass
    import concourse.mybir as mybir
    import concourse.tile as tile

    dt = mybir.dt
    AF = mybir.ActivationFunctionType
    OP = mybir.AluOpType

    nc = bass.Bass("TRN2", target_bir_lowering=False, debug=False,
                   num_devices=N_CORES)

    xT_d = nc.dram_tensor("xT", [IN_FEAT, FD], dt.bfloat16, kind="ExternalInput")
    T_d = nc.dram_tensor("Tm", [IN_FEAT, OK], dt.bfloat16, kind="ExternalInput")
    sel_d = nc.dram_tensor("sel", [128, NT * SELW], dt.bfloat16,
                           kind="ExternalInput")
    sel2_d = nc.dram_tensor("sel2", [128, OUT_FEAT], dt.bfloat16,
                            kind="ExternalInput")
    selS_d = nc.dram_tensor("selS", [128, NT * SELW], dt.bfloat16,
                            kind="ExternalInput")
    dneg_d = nc.dram_tensor("dneg", [OUT_FEAT, OUT_FEAT], dt.bfloat16,
                            kind="ExternalInput")
    rows_d = nc.dram_tensor("rowS", [OUT_FEAT, ROWB], dt.float32,
                            kind="ExternalOutput")
    acc_d = nc.dram_tensor("accS", [OUT_FEAT, FD], dt.float32,
                           kind="ExternalOutput")

    with tile.TileContext(nc) as tc, ExitStack() as ctx:
        const = ctx.enter_context(tc.tile_pool(name="const", bufs=1))
        mtp = ctx.enter_context(tc.tile_pool(name="mt", bufs=NT))
        psA = ctx.enter_context(
            tc.tile_pool(name="psA", bufs=1, space=bass.MemorySpace.PSUM))
        psDA = ctx.enter_context(
            tc.tile_pool(name="psDA", bufs=3, space=bass.MemorySpace.PSUM))
        psDB = ctx.enter_context(
            tc.tile_pool(name="psDB", bufs=2, space=bass.MemorySpace.PSUM))
        psC = ctx.enter_context(
            tc.tile_pool(name="psC", bufs=1, space=bass.MemorySpace.PSUM))
        workV = ctx.enter_context(tc.tile_pool(name="workV", bufs=ADV_BUFS))
        # ACT-written tiles (Abs elementwise + Exp outputs) share one pool:
        # the WAW chain keeps ACT's scheduled order near program order
        ep = ctx.enter_context(tc.tile_pool(name="e", bufs=64))
        # fold outputs are fresh (never reused) so the fold and the Exp that
        # reads it each carry exactly one wait
        foldp = ctx.enter_context(tc.tile_pool(name="fold", bufs=ROWB))

        Tsb = []
        for kc in range(4):
            t_ = const.tile([128, OK], dt.bfloat16, tag=f"T{kc}")
            nc.sync.dma_start(t_[:], T_d[kc * 128:(kc + 1) * 128, :])
            Tsb.append(t_)
        xTsb = []
        for kc in range(4):
            t_ = const.tile([128, FD], dt.bfloat16, tag=f"x{kc}")
            nc.sync.dma_start(t_[:], xT_d[kc * 128:(kc + 1) * 128, :])
            xTsb.append(t_)
        sel = const.tile([128, NT * SELW], dt.bfloat16, tag="sel")
        nc.sync.dma_start(sel[:], sel_d[:])
        sel2 = const.tile([128, OUT_FEAT], dt.bfloat16, tag="sel2")
        nc.sync.dma_start(sel2[:], sel2_d[:])
        selS = const.tile([128, NT * SELW], dt.bfloat16, tag="selS")
        nc.sync.dma_start(selS[:], selS_d[:])
        dneg = const.tile([OUT_FEAT, OUT_FEAT], dt.bfloat16, tag="dneg")
        nc.sync.dma_start(dneg[:], dneg_d[:])
        mcol = const.tile([128, NT * ROWB], dt.float32, tag="mcol")
        rowS = const.tile([OUT_FEAT, ROWB], dt.float32, tag="rowS")
        accS = const.tile([OUT_FEAT, FD], dt.float32, tag="accS")
        nc.vector.memset(accS[:], 0.0)

        # Mt tiles: Mt[(o,k), j], tile t holds o in [4t, 4t+4), all k
        r_tiles = [t for t in range(NT) if t not in ACT_TILES]
        psS = psA.tile([OUT_FEAT, FD], dt.float32, tag="psS")
        mts = []
        for t in range(NT):
            ps = psA.tile([128, FD], dt.float32)
            for kc in range(4):
                nc.tensor.matmul(ps[:],
                                 Tsb[kc][:, t * 128:(t + 1) * 128],
                                 xTsb[kc][:],
                                 start=(kc == 0), stop=(kc == 3))
            mt_t = mtp.tile([128, FD], dt.bfloat16, tag="mt")
            nc.vector.tensor_copy(mt_t[:], ps[:])
            # scalar columns: the *rounded* bf16 values recast to fp32 so the
            # diagonal difference is exactly zero
            nc.vector.tensor_copy(mcol[:, t * ROWB:(t + 1) * ROWB],
                                  mt_t[:, 0:ROWB])
            mts.append(mt_t)

        # S[o, j] = sum_k Mt[(o,k), j] over the relu tiles only; kept in
        # bf16 so the Exp bias cancels the matmul term exactly on the
        # diagonal: D_ii = 2*0 + S_i - S_i = 0.
        for m, t in enumerate(r_tiles):
            nc.tensor.matmul(psS[:], selS[:, t * SELW:(t + 1) * SELW],
                             mts[t][:], start=(m == 0),
                             stop=(m == len(r_tiles) - 1))
        S_bf = const.tile([OUT_FEAT, FD], dt.bfloat16, tag="S_bf")
        nc.vector.tensor_copy(S_bf[:], psS[:])
        Sneg = const.tile([OUT_FEAT, ROWB], dt.float32, tag="Sneg")
        nc.vector.tensor_scalar(Sneg[:], S_bf[:, 0:ROWB], -1.0, None,
                                op0=OP.mult)
        # warm up ACT's observed DVE clock so the first ACT op (reading
        # DVE-written tiles) does not need a second sync wait
        warmA = const.tile([1, 1], dt.float32, tag="warmA")
        nc.scalar.copy(warmA[:], Sneg[0:1, 0:1])

        e_tiles_of = {}          # chunk -> list of packed e tiles
        pending = None           # (psd2, i) awaiting fold+exp

        def fold_exp(pair, i):
            psda, psdb = pair
            # move the B half to SBUF (bf16) and fold it into the A bank on
            # PE via an identity matmul (sel2's top half is I64); the copy
            # doubles as DVE's PE-clock refresh
            b_sb = foldp.tile([OUT_FEAT, FD], dt.bfloat16, tag="fold",
                              name=f"bsb_{i}")
            nc.vector.tensor_copy(b_sb[:], psdb[OUT_FEAT:128, :])
            nc.tensor.matmul(psda[:], sel2[0:OUT_FEAT, :],
                             b_sb[:], start=False, stop=True)
            ch = i // CHUNK
            if i % 2 == 0:
                e_t = ep.tile([128, FD], dt.bfloat16, tag="e",
                              name=f"e_{i}")
                e_tiles_of.setdefault(ch, []).append(e_t)
            half = e_tiles_of[ch][-1][(i % 2) * OUT_FEAT:
                                      (i % 2 + 1) * OUT_FEAT, :]
            nc.scalar.activation(half, psda[:], AF.Exp,
                                 scale=-1.0, bias=Sneg[:, i:i + 1],
                                 accum_out=rowS[:, i:i + 1])
            if i % CHUNK == CHUNK - 1:
                # column-sum partials for this chunk
                psc = psC.tile([OUT_FEAT, FD], dt.float32)
                ets = e_tiles_of[ch]
                for m, e_t in enumerate(ets):
                    nc.tensor.matmul(psc[:], sel2[:], e_t[:],
                                     start=(m == 0),
                                     stop=(m == len(ets) - 1))
                nc.vector.tensor_add(accS[:], accS[:], psc[:])

        for i in range(ROWB):
            psda = psDA.tile([OUT_FEAT, FD], dt.float32, tag="psda",
                             name=f"psda_{i}")
            psdb = psDB.tile([128, FD], dt.float32, tag="psdb",
                             name=f"psdb_{i}")
            ads = {}
            for t in range(NT):
                sc = mcol[:, t * ROWB + i: t * ROWB + i + 1]
                if t in ACT_TILES:
                    ad_t = ep.tile([128, FD], dt.bfloat16, tag="e",
                                   name=f"adA_{i}_{t}")
                    nc.scalar.activation(ad_t[:], mts[t][:], AF.Abs,
                                         bias=sc, scale=-1.0)
                else:
                    ad_t = workV.tile([128, FD], dt.bfloat16, tag="adV",
                                      name=f"ad_{i}_{t}")
                    nc.vector.tensor_scalar(ad_t[:], mts[t][:], sc, 0.0,
                                            op0=OP.subtract, op1=OP.max)
                ads[t] = ad_t
            # 8 column-tiled matmul pairs: even tile -> partitions 0:64
            # (array col group 0-1), odd tile -> 64:128 (col group 2-3);
            # the two streams use disjoint column groups and overlap
            for p in range(NT // 2):
                nc.tensor.matmul(psda[:],
                                 sel[:, (2 * p) * SELW:(2 * p + 1) * SELW],
                                 ads[2 * p][:],
                                 start=(p == 0), stop=False)
                nc.tensor.matmul(psdb[OUT_FEAT:128, :],
                                 sel[:, (2 * p + 1) * SELW:
                                      (2 * p + 2) * SELW],
                                 ads[2 * p + 1][:],
                                 start=(p == 0), stop=(p == NT // 2 - 1),
                                 tile_position=(0, 64))
            # -S_j correction joins the A bank (group stays open: the
            # fold matmul emitted next iteration closes it)
            nc.tensor.matmul(psda[:], dneg[:], S_bf[:],
                             start=False, stop=False)
            # fold+exp of the PREVIOUS iteration: its PE wait is already
            # satisfied, so the DVE never stalls, and it refreshes DVE's
            # observed PE clock for the elementwise slot reuse
            if pending is not None:
                fold_exp(*pending)
            pending = ((psda, psdb), i)
        fold_exp(*pending)

        # outputs go out on the SW-DGE queues (gpsimd): the HW-DGE queues
        # carried the input loads, and a shared queue would add a second
        # sync-wait command that the DMA pseudo-instruction cannot encode
        nc.gpsimd.dma_start(rows_d[:], rowS[:])
        nc.gpsimd.dma_start(acc_d[:], accS[:])

    if split_waits:
        _split_multiwaits(nc, mybir)
    return nc


def _split_multiwaits(nc, mybir):
    """Walrus on this toolchain encodes at most ONE sync-wait command per
    instruction.  Split any instruction with more waits (in practice only
    the framework's kernel-tail drain) into a chain of single-wait Drain
    carriers on the same engine, inserted immediately before it."""
    n = 0
    for fn in nc.m.functions:
        for bb in fn.blocks:
            new_insts = []
            for inst in bb.instructions:
                si = getattr(inst, "sync_info", None)
                if si is not None and si.on_wait and len(si.on_wait) > 1:
                    waits = list(si.on_wait)
                    for w in waits[:-1]:
                        carrier = mybir.InstDrain(
                            name=f"splitw_{n}", engine=inst.engine,
                            ins=[], outs=[],
                            sync_info=mybir.SyncInfo(on_wait=[w],
                                                     on_update=[]))
                        new_insts.append(carrier)
                        n += 1
                    inst.sync_info = mybir.SyncInfo(
                        on_wait=[waits[-1]], on_update=list(si.on_update))
                new_insts.append(inst)
            if n:
                bb.instructions = new_insts


def _sel_host(value, act_value=None):
    sel = np.zeros((128, NT * SELW), dtype=np.float32)
    for t in range(NT):
        v = value if (act_value is None or t not in ACT_TILES) else act_value
        for g in range(4):
            sel[32 * g:32 * (g + 1), t * SELW + 4 * t + g] = v
    return sel.astype(ml_dtypes.bfloat16)


def _sel2_host():
    s = np.zeros((128, OUT_FEAT), dtype=np.float32)
    s[:OUT_FEAT, :] = np.eye(OUT_FEAT)
    s[OUT_FEAT:, :] = np.eye(OUT_FEAT)
    return s.astype(ml_dtypes.bfloat16)


def _block_order(c):
    """Column blocks for core c; None marks the poison block."""
    if c < 4:
        return [c, c + 1, c + 2, c + 3, c + 4]
    return [c, (c + 1) % 8, (c + 2) % 8, (c + 3) % 8, None]


def _in_maps(x, T):
    bf16 = ml_dtypes.bfloat16
    Tb = np.ascontiguousarray(T.reshape(IN_FEAT, OK)).astype(bf16)
    selb = _sel_host(2.0, act_value=1.0)
    selSb = _sel_host(1.0)
    sel2b = _sel2_host()
    dnegb = (-np.eye(OUT_FEAT, dtype=np.float32)).astype(bf16)
    xT = np.ascontiguousarray(x.T)
    maps = []
    for c in range(N_CORES):
        xTc = np.empty((IN_FEAT, FD), dtype=np.float32)
        for pos, b in enumerate(_block_order(c)):
            if b is None:
                xTc[:, 64 * pos:64 * (pos + 1)] = POISON
            else:
                xTc[:, 64 * pos:64 * (pos + 1)] = xT[:, 64 * b:64 * (b + 1)]
        maps.append({"xT": xTc.astype(bf16), "Tm": Tb, "sel": selb,
                     "selS": selSb, "sel2": sel2b, "dneg": dnegb})
    return maps


def kernel(x, T):
    from concourse import bass_utils

    x = np.asarray(x, dtype=np.float32)
    T = np.asarray(T, dtype=np.float32)

    if "nc" not in _cache:
        _cache["nc"] = _build_nc()
    nc = _cache["nc"]

    res = bass_utils.run_bass_kernel_spmd(
        nc, _in_maps(x, T), core_ids=list(range(N_CORES)))

    mbd = np.zeros((BATCH, OUT_FEAT), dtype=np.float32)
    for c in range(N_CORES):
        rs = np.asarray(res.results[c]["rowS"], dtype=np.float32)  # [o, i]
        mbd[64 * c:64 * (c + 1), :] += rs.T
        acc = np.asarray(res.results[c]["accS"], dtype=np.float32)  # [o, j]
        for pos, b in enumerate(_block_order(c)):
            if pos == 0 or b is None:
                continue  # own diag block is fully in rowsums; poison dropped
            mbd[64 * b:64 * (b + 1), :] += acc[:, 64 * pos:64 * (pos + 1)].T
    mbd -= 1.0
    return np.concatenate([x, mbd], axis=1)



# revision 58
# speedup vs baseline: 1.1531x; 1.1531x over previous
"""MiniBatchDiscrimination kernel, v5: relu elementwise on three engines
+ 4-way column-tiled PE streams + paired-row Exp.

Math per core (row block of 64 i's x FD=320 j columns spanning 5 blocks):
  Mt[(o,k), j] = M^T in bf16 (16 partition-tiles of 128 = 4 o x 32 k),
  computed on PE from T and x^T.
  For each i: relu tiles R_t = max(Mt_t - Mt_t[:, i], 0) on DVE (11) and
    GpSimd (2); abs tiles |Mt_t - Mt_t[:, i]| on ACT (3) via
    activation(Abs, scale=-1, bias).  The scalar column is the bf16 Mt
    value recast to fp32 (mcol), so the diagonal difference is exactly 0.
  D[o, j] = 2*sum_k relu + sum_k |d| - (S_j - S_i): the k-reduction is 16
    matmuls per i with selection weights (2.0 relu / 1.0 abs tiles, 4
    nonzero output rows each); -S_j lands first via one dmap matmul per
    PAIR covering all partitions; +S_i rides the Exp bias (Sneg2).
  Two i's share one PSUM bank: even i -> partitions 0:64, odd -> 64:128,
  each split again into o-halves -> FOUR concurrent column-tiled PE
  streams (tile_position (0,0)/(0,32)/(0,64)/(0,96), 32-wide weights).
  The bank is reset by a tiny start=True matmul into its top 2 columns
  (start marks the whole 2KB zero region pending-zero), so the dmap
  matmul overwrites and the streams accumulate with start=False.
  One Exp per PAIR of i's: activation [128, 320] with accum_out giving
  both rowsums; column-sum partials accumulate in a persistent PSUM bank
  via one matmul per two pairs (a DVE add folds two exp tiles first).

Sharding (unchanged from v4): symmetric-pair blocks, 5 column blocks per
core (cores 4-7 carry one poisoned block); host adds row- and mirrored
column-sums and subtracts the self-similarity 1.
"""

import numpy as np
import ml_dtypes
from contextlib import ExitStack

BATCH, IN_FEAT, OUT_FEAT, KERNEL_DIM = 512, 512, 64, 32
N_CORES = 8
ROWB = BATCH // N_CORES          # 64 rows of i per core
NPAIR = ROWB // 2                # 32 exp/psum groups
OK = OUT_FEAT * KERNEL_DIM       # 2048 flattened (o,k)
NT = OK // 128                   # 16 partition-tiles of (o,k)
NBLK = 5                         # column blocks per core
FD = NBLK * 64                   # 320
POISON = 1.0e4

SELW = 32                        # per-tile weight width (o-half streams)
ACT_TILES = (5, 10, 13)          # elementwise tiles computed on ACT as Abs
POOL_TILES = (2, 7)              # elementwise tiles computed on GpSimd
DVE_BUFS = 48
ACT_BUFS = 20
POOL_BUFS = 12
# NOTE: the walrus ISA rejects abs_max on InstTensorScalarPtr (probed:
# every variant), so the DVE/Pool tiles compute relu(d) (weight 2.0) and
# the missing -d term is restored per pair by one -S_j matmul (dmap)
# plus the +S_i exp bias: sum|d| = 2*sum relu(d) - (S_j - S_i).

_cache = {}


def _build_nc(split_waits=True):
    import concourse.bass as bass
    import concourse.mybir as mybir
    import concourse.tile as tile

    dt = mybir.dt
    AF = mybir.ActivationFunctionType
    OP = mybir.AluOpType

    nc = bass.Bass("TRN2", target_bir_lowering=False, debug=False,
                   num_devices=N_CORES)

    xT_d = nc.dram_tensor("xT", [IN_FEAT, FD], dt.bfloat16, kind="ExternalInput")
    T_d = nc.dram_tensor("Tm", [IN_FEAT, OK], dt.bfloat16, kind="ExternalInput")
    sel_d = nc.dram_tensor("sel", [128, NT * SELW], dt.bfloat16,
                           kind="ExternalInput")
    sel2_d = nc.dram_tensor("sel2", [128, OUT_FEAT], dt.bfloat16,
                            kind="ExternalInput")
    dmap_d = nc.dram_tensor("dmap", [OUT_FEAT, 128], dt.bfloat16,
                            kind="ExternalInput")
    rows_d = nc.dram_tensor("rowS2", [128, NPAIR], dt.float32,
                            kind="ExternalOutput")
    acc_d = nc.dram_tensor("accS", [OUT_FEAT, FD], dt.float32,
                           kind="ExternalOutput")
    # the last pair's exp tile goes out raw; the host folds it into the
    # column sums, keeping the final colsum+copy+DMA off the kernel tail
    eL_d = nc.dram_tensor("eLast", [128, FD], dt.bfloat16,
                          kind="ExternalOutput")

    with tile.TileContext(nc) as tc, ExitStack() as ctx:
        const = ctx.enter_context(tc.tile_pool(name="const", bufs=1))
        mtp = ctx.enter_context(tc.tile_pool(name="mt", bufs=NT))
        psA = ctx.enter_context(
            tc.tile_pool(name="psA", bufs=2, space=bass.MemorySpace.PSUM))
        psWp = ctx.enter_context(
            tc.tile_pool(name="psW", bufs=1, space=bass.MemorySpace.PSUM))
        psSp = ctx.enter_context(
            tc.tile_pool(name="psS", bufs=1, space=bass.MemorySpace.PSUM))
        psD = ctx.enter_context(
            tc.tile_pool(name="psD", bufs=3, space=bass.MemorySpace.PSUM))
        psC = ctx.enter_context(
            tc.tile_pool(name="psC", bufs=1, space=bass.MemorySpace.PSUM))
        workV = ctx.enter_context(tc.tile_pool(name="workV", bufs=DVE_BUFS))
        workP = ctx.enter_context(tc.tile_pool(name="workP", bufs=POOL_BUFS))
        # ACT-written tiles (Abs elementwise + Exp outputs) share one pool:
        # the WAW chain keeps ACT's scheduled order near program order
        ep = ctx.enter_context(tc.tile_pool(name="e", bufs=ACT_BUFS))

        # inputs split between the two HWDGE queues (SP + ACT) and
        # interleaved T/xT so the first M-setup matmuls start early
        # T chunks on the SP queue, xT/sel on the ACT queue: the DGE
        # round-robins the queues, interleaving each T chunk with the xT
        # it is contracted against
        Tsb, xTsb = [], []
        for kc in range(4):
            t_ = const.tile([128, OK], dt.bfloat16, tag=f"T{kc}")
            nc.sync.dma_start(t_[:], T_d[kc * 128:(kc + 1) * 128, :])
            Tsb.append(t_)
            x_ = const.tile([128, FD], dt.bfloat16, tag=f"x{kc}")
            nc.scalar.dma_start(x_[:], xT_d[kc * 128:(kc + 1) * 128, :])
            xTsb.append(x_)
        sel = const.tile([128, NT * SELW], dt.bfloat16, tag="sel")
        nc.sync.dma_start(sel[:], sel_d[:])
        sel2 = const.tile([128, OUT_FEAT], dt.bfloat16, tag="sel2")
        nc.sync.dma_start(sel2[:], sel2_d[:])
        dmap = const.tile([OUT_FEAT, 128], dt.bfloat16, tag="dmap")
        nc.sync.dma_start(dmap[:], dmap_d[:])
        # 1x128 zero weight: a K=1 matmul with it writes 0 to a whole PSUM
        # bank, resetting values + has_written in one cheap PE instruction
        zeroW = const.tile([1, 128], dt.bfloat16, tag="zeroW")
        nc.vector.memset(zeroW[:], 0.0)
        # two tiles so the first half's DMA only waits on exps 0..15
        rowS2a = const.tile([128, NPAIR // 2], dt.float32, tag="rowS2a")
        rowS2b = const.tile([128, NPAIR // 2], dt.float32, tag="rowS2b")
        accS = const.tile([OUT_FEAT, FD], dt.float32, tag="accS")
        # scalar columns: the *rounded* bf16 values recast to fp32 so the
        # diagonal difference is exactly zero.  One tile per consuming
        # engine, written BY that engine, so each absdiff's scalar read
        # needs no cross-engine wait
        n_dve = NT - len(ACT_TILES) - len(POOL_TILES)
        mcolV = const.tile([128, n_dve * ROWB], dt.float32, tag="mcolV")
        mcolA = const.tile([128, len(ACT_TILES) * ROWB], dt.float32,
                           tag="mcolA")
        mcolP = const.tile([128, len(POOL_TILES) * ROWB], dt.float32,
                           tag="mcolP")
        mcol_of = {}
        for t in range(NT):
            if t in ACT_TILES:
                mcol_of[t] = (mcolA, ACT_TILES.index(t))
            elif t in POOL_TILES:
                mcol_of[t] = (mcolP, POOL_TILES.index(t))
            else:
                dv = [u for u in range(NT)
                      if u not in ACT_TILES and u not in POOL_TILES]
                mcol_of[t] = (mcolV, dv.index(t))

        # warm the PE clock (HAM / p-state) during the input DMA window
        # with zero matmuls on the already-memset zeroW tile
        psW = psWp.tile([128, 128], dt.float32, tag="psW")
        for w in range(50):
            nc.tensor.matmul(psW[:], zeroW[:], zeroW[:],
                             start=True, stop=True)

        # Mt tiles: Mt[(o,k), j], tile t holds o in [4t, 4t+4), all k.
        # PSUM->SBUF bf16 copies split between DVE and ACT.
        mts = [None] * NT
        for t in [v for p in range(NT // 2) for v in (p, p + NT // 2)]:
            ps = psA.tile([128, FD], dt.float32)
            for kc in range(4):
                nc.tensor.matmul(ps[:],
                                 Tsb[kc][:, t * 128:(t + 1) * 128],
                                 xTsb[kc][:],
                                 start=(kc == 0), stop=(kc == 3))
            mt_t = mtp.tile([128, FD], dt.bfloat16, tag="mt")
            if t % 2 == 0:
                nc.vector.tensor_copy(mt_t[:], ps[:])
            else:
                nc.scalar.copy(mt_t[:], ps[:])
            mc, ci = mcol_of[t]
            dst = mc[:, ci * ROWB:(ci + 1) * ROWB]
            if t in ACT_TILES:
                nc.scalar.copy(dst, mt_t[:, 0:ROWB])
            elif t in POOL_TILES:
                nc.gpsimd.tensor_copy(dst, mt_t[:, 0:ROWB])
            else:
                nc.vector.tensor_copy(dst, mt_t[:, 0:ROWB])
            mts[t] = mt_t

        lo_tiles = [t for t in range(NT // 2)]          # o in [0, 32)
        hi_tiles = [t for t in range(NT // 2, NT)]      # o in [32, 64)

        # S[o, j] = sum_k Mt[(o,k), j] over the relu tiles (selS is zero on
        # the ACT tiles' o's), in bf16 so the exp bias cancels the -S_j
        # matmul exactly on the diagonal
        r_tiles = [t for t in range(NT) if t not in ACT_TILES]
        psS = psSp.tile([OUT_FEAT, 512], dt.float32, tag="psS")
        nc.tensor.matmul(psS[:, 510:512], zeroW[0:1, 0:OUT_FEAT],
                         sel[0:1, 0:2], start=True, stop=True)
        for t in r_tiles:
            oh = 0 if t < NT // 2 else 32
            nc.tensor.matmul(psS[oh:oh + 32, 0:FD],
                             sel[:, t * SELW:(t + 1) * SELW], mts[t][:],
                             start=False, stop=False, skip_group_check=True,
                             tile_position=(0, oh))
        # sel carries the relu weight 2.0; halve while converting to bf16
        # (on ACT: DVE is the busier engine during the pipeline fill)
        S_bf = const.tile([OUT_FEAT, FD], dt.bfloat16, tag="S_bf")
        nc.scalar.activation(S_bf[:], psS[:, 0:FD], AF.Copy, scale=0.5)
        # exp bias: Sneg2[p, m] = -S[o(p), 2m + (p // 64)]
        Sneg2 = const.tile([128, NPAIR], dt.float32, tag="Sneg2")
        nc.scalar.activation(Sneg2[0:OUT_FEAT, :], S_bf[:, 0:2 * NPAIR:2],
                             AF.Copy, scale=-1.0)
        nc.scalar.activation(Sneg2[OUT_FEAT:128, :], S_bf[:, 1:2 * NPAIR:2],
                             AF.Copy, scale=-1.0)

        def absdiff(t, i, name):
            mc, ci = mcol_of[t]
            sc = mc[:, ci * ROWB + i: ci * ROWB + i + 1]
            if t in ACT_TILES:
                ad_t = ep.tile([128, FD], dt.bfloat16, tag="e", name=name)
                nc.scalar.activation(ad_t[:], mts[t][:], AF.Abs,
                                     bias=sc, scale=-1.0)
            elif t in POOL_TILES:
                ad_t = workP.tile([128, FD], dt.bfloat16, tag="adP",
                                  name=name)
                nc.gpsimd.tensor_scalar(ad_t[:], mts[t][:], sc, 0.0,
                                        op0=OP.subtract, op1=OP.max)
            else:
                ad_t = workV.tile([128, FD], dt.bfloat16, tag="adV",
                                  name=name)
                nc.vector.tensor_scalar(ad_t[:], mts[t][:], sc, 0.0,
                                        op0=OP.subtract, op1=OP.max)
            return ad_t

        psc = psC.tile([OUT_FEAT, FD], dt.float32, tag="psc")

        def exp_pair(psd, m, accum=True):
            e_t = ep.tile([128, FD], dt.bfloat16, tag="e", name=f"e_{m}")
            half, col = divmod(m, NPAIR // 2)
            rs = rowS2b if half else rowS2a
            kw = {"accum_out": rs[:, col:col + 1]} if accum else {}
            nc.scalar.activation(e_t[:], psd[:, 0:FD], AF.Exp, scale=-1.0,
                                 bias=Sneg2[:, m:m + 1], **kw)
            return e_t

        def colsum(e_a, e_b, q):
            # fold two pairs' exp tiles on DVE, halving the colsum matmuls
            es = workV.tile([128, FD], dt.bfloat16, tag="adV",
                            name=f"esum_{q}")
            nc.vector.tensor_add(es[:], e_a[:], e_b[:])
            nc.tensor.matmul(psc[:], sel2[:], es[:],
                             start=(q == 0), stop=(q == NPAIR // 2 - 1))

        pending = None           # (psd, m) awaiting exp
        pending_e = []           # e tiles awaiting colsum matmul

        for m in range(NPAIR):
            last = m == NPAIR - 1
            iA, iB = 2 * m, 2 * m + 1
            ads = {}
            for p in range(NT // 2):
                for ih, i in ((0, iA), (1, iB)):
                    for t in (lo_tiles[p], hi_tiles[p]):
                        ads[(t, ih)] = absdiff(t, i, f"ad_{m}_{t}_{ih}")
            if last:
                # emit exp(30) before the last pair's matmuls so e30's
                # colsum can slot into the middle of the stream block
                e_prev = exp_pair(*pending)
                pending = None
            # full-bank tile: columns 0:FD carry D.  The dmap matmul both
            # resets the bank and writes -S_j for all four quarters:
            # start=True marks the whole 2KB zero region pending-zero (its
            # own write then lands as an overwrite), stop=True closes the
            # sim's group so the next pair can start; the streams then
            # accumulate with start=False.
            psd = psD.tile([128, 512], dt.float32, tag="psd",
                           name=f"psd_{m}")
            nc.tensor.matmul(psd[:, 0:FD], dmap[:], S_bf[:],
                             start=True, stop=True)
            # 4 column-tiled streams: array col quarter = 64*i + 32*o_half
            for p in range(NT // 2):
                if last and p == 4:
                    # e30 closes the psc group mid-stream: the accS
                    # copies + DMA then overlap the rest of the pair
                    nc.tensor.matmul(psc[:], sel2[:], e_prev[:],
                                     start=False, stop=True)
                for ih in (0, 1):
                    for oh, tlist in ((0, lo_tiles), (1, hi_tiles)):
                        t = tlist[p]
                        cp = 64 * ih + 32 * oh
                        nc.tensor.matmul(
                            psd[cp:cp + 32, 0:FD],
                            sel[:, t * SELW:(t + 1) * SELW],
                            ads[(t, ih)][:],
                            start=False, stop=False,
                            skip_group_check=True,
                            tile_position=(0, cp))
            if last:
                HF = FD // 2
                nc.vector.tensor_copy(accS[:, 0:HF], psc[:, 0:HF])
                nc.scalar.copy(accS[:, HF:FD], psc[:, HF:FD])
                nc.sync.dma_start(acc_d[:], accS[:])
                # rowsums of exps 16..30; the last pair's come from eLast
                nc.sync.dma_start(rows_d[:, NPAIR // 2:NPAIR - 1],
                                  rowS2b[:, 0:NPAIR // 2 - 1])
            # exp of the PREVIOUS pair: its PE wait is already satisfied,
            # so ACT never stalls; colsum trails two pairs behind
            if pending is not None:
                pending_e.append(exp_pair(*pending))
                if len(pending_e) == 2:
                    colsum(*pending_e, pending[1] // 2)
                    pending_e = []
                if pending[1] == NPAIR // 2 - 1:
                    # first half of the rowsums is final: ship it early
                    nc.sync.dma_start(rows_d[:, 0:NPAIR // 2], rowS2a[:])
            pending = (psd, m)
        # tail: exp31's tile ships raw; the host folds it into the column
        # sums and derives the last two rowsums from it
        e_last = exp_pair(*pending, accum=False)
        assert not pending_e
        nc.scalar.dma_start(eL_d[:], e_last[:])

    if split_waits:
        _split_multiwaits(nc, mybir)
    return nc


def _split_multiwaits(nc, mybir):
    """Walrus on this toolchain encodes at most ONE sync-wait command per
    instruction.  Split any instruction with more waits into a chain of
    single-wait Drain carriers on the same engine, inserted immediately
    before it."""
    n = 0
    for fn in nc.m.functions:
        for bb in fn.blocks:
            new_insts = []
            for inst in bb.instructions:
                si = getattr(inst, "sync_info", None)
                if si is not None and si.on_wait and len(si.on_wait) > 1:
                    waits = list(si.on_wait)
                    for w in waits[:-1]:
                        carrier = mybir.InstDrain(
                            name=f"splitw_{n}", engine=inst.engine,
                            ins=[], outs=[],
                            sync_info=mybir.SyncInfo(on_wait=[w],
                                                     on_update=[]))
                        new_insts.append(carrier)
                        n += 1
                    inst.sync_info = mybir.SyncInfo(
                        on_wait=[waits[-1]], on_update=list(si.on_update))
                new_insts.append(inst)
            if n:
                bb.instructions = new_insts


def _sel_host():
    """Selection weights: tile t's partition group g (o = 4t+g, 32 k's)
    sums into weight column (4t+g) mod 32 of its o-half stream, scaled
    2.0 for relu tiles and 1.0 for the ACT |d| tiles."""
    sel = np.zeros((128, NT * SELW), dtype=np.float32)
    for t in range(NT):
        v = 1.0 if t in ACT_TILES else 2.0
        for g in range(4):
            sel[32 * g:32 * (g + 1), t * SELW + (4 * t + g) % SELW] = v
    return sel.astype(ml_dtypes.bfloat16)


def _dmap_host():
    """-S broadcast: output partition p of a pair bank holds (i-half
    p//64, o = 32*((p//32)%2) + p%32) and receives -S[o, j]."""
    d = np.zeros((OUT_FEAT, 128), dtype=np.float32)
    for p in range(128):
        o = 32 * ((p // 32) % 2) + p % 32
        d[o, p] = -1.0
    return d.astype(ml_dtypes.bfloat16)


def _sel2_host():
    s = np.zeros((128, OUT_FEAT), dtype=np.float32)
    s[:OUT_FEAT, :] = np.eye(OUT_FEAT)
    s[OUT_FEAT:, :] = np.eye(OUT_FEAT)
    return s.astype(ml_dtypes.bfloat16)


def _block_order(c):
    """Column blocks for core c; None marks the poison block."""
    if c < 4:
        return [c, c + 1, c + 2, c + 3, c + 4]
    return [c, (c + 1) % 8, (c + 2) % 8, (c + 3) % 8, None]


def _in_maps(x, T):
    bf16 = ml_dtypes.bfloat16
    Tb = np.ascontiguousarray(T.reshape(IN_FEAT, OK)).astype(bf16)
    selb = _sel_host()
    sel2b = _sel2_host()
    dmapb = _dmap_host()
    xT = np.ascontiguousarray(x.T)
    maps = []
    for c in range(N_CORES):
        xTc = np.empty((IN_FEAT, FD), dtype=np.float32)
        for pos, b in enumerate(_block_order(c)):
            if b is None:
                xTc[:, 64 * pos:64 * (pos + 1)] = POISON
            else:
                xTc[:, 64 * pos:64 * (pos + 1)] = xT[:, 64 * b:64 * (b + 1)]
        maps.append({"xT": xTc.astype(bf16), "Tm": Tb, "sel": selb,
                     "sel2": sel2b, "dmap": dmapb})
    return maps


def _gather(results):
    """results: per-core dict with rowS2 [128, NPAIR], accS [64, FD] and
    eLast [128, FD] (the last pair's raw exp tile, folded here)."""
    mbd = np.zeros((BATCH, OUT_FEAT), dtype=np.float32)
    for c in range(N_CORES):
        rs = np.array(results[c]["rowS2"], dtype=np.float32)
        eL = np.asarray(results[c]["eLast"], dtype=np.float32)
        rs[:, NPAIR - 1] = eL.sum(axis=1)
        # partitions [64s:64s+64] of column m are the rowsum of i = 2m+s
        rows = rs.reshape(2, OUT_FEAT, NPAIR).transpose(2, 0, 1)
        mbd[64 * c:64 * (c + 1), :] += rows.reshape(ROWB, OUT_FEAT)
        acc = np.asarray(results[c]["accS"], dtype=np.float32)  # [o, j]
        acc = acc + eL[0:OUT_FEAT] + eL[OUT_FEAT:128]
        for pos, b in enumerate(_block_order(c)):
            if pos == 0 or b is None:
                continue  # own diag block is fully in rowsums; poison dropped
            mbd[64 * b:64 * (b + 1), :] += acc[:, 64 * pos:64 * (pos + 1)].T
    mbd -= 1.0
    return mbd


def kernel(x, T):
    from concourse import bass_utils

    x = np.asarray(x, dtype=np.float32)
    T = np.asarray(T, dtype=np.float32)

    if "nc" not in _cache:
        _cache["nc"] = _build_nc()
    nc = _cache["nc"]

    res = bass_utils.run_bass_kernel_spmd(
        nc, _in_maps(x, T), core_ids=list(range(N_CORES)))

    mbd = _gather(res.results)
    return np.concatenate([x, mbd], axis=1)
